# revision 43
# baseline (speedup 1.0000x reference)
"""Trainium2 Bass kernel for nn_MHAttentionLayer_64587718197528.

Reference computation (B=4, L=1024, D_MODEL=1024, S=2048, T=NUM_TOKENS=1000,
H=16, E=256, D_LLM=4096):
    q = (X @ Wq.T + bq)            [B*L, H*E]      X = target_embedding
    k = (SE @ Wk.T + bk)           [S, H*E]        SE = source_embedding
    v = (VE @ Wv.T + bv)           [S, H*E]        VE = value_embedding
    scores[b,h,l,s] = q . k / 16 ; A = softmax_s ; out = A @ v
    y = out @ Wo.T + bo            [B*L, D_LLM]

Sharding: tensor-parallel over heads. Core i owns heads {2i, 2i+1} (an
e-slice of 512 of the H*E dim). Each core computes its q/k/v projections,
attention for its 2 heads, and a partial out-projection
  partial_i = attn_out_i @ Wo[:, sl_i].T          [B*L, D_LLM]
The host sums the 8 partials and adds bo (linearity of the projection).

All matmul operands are bf16 (full PE rate; ~1e-3 rel err, gate is 2e-2).
The T (=1000/1001) contraction dim is zero-padded to 1024 host-side so all
k-tiles are a uniform 128 rows. Output partials are written bf16 (halves
the 64MB/core output stream). Phases:
  KV:   kT[512,2048] = Wk_i @ SE.T and v[2048,512] = VE_aug @ Wv_aug
        (bias for v folded via ones-row augmentation), SBUF-resident.
        Wq and the whole Wo are prefetched during this phase.
  Attn: per l-chunk of 512: q-projection (inputs prefetched one chunk
        ahead; the two accumulation chains per half run back-to-back so
        their PSUM evictions overlap the next chain), scoresT[s,l] in
        double-buffered PSUM, exp on ACT (scale=1/16; no max subtraction
        -- |scaled scores| < ~8) with one a-tile per score step (avoids
        tile-granular false WARs), AV matmuls software-pipelined
        AV_DELAY steps behind the scores so PE never waits on ACT
        latency, softmax denominators via DVE accumulation (bf16 fold,
        so the ones-matmul partition-reduce stays in bf16 -- no PE
        fp32-mode switch) + reciprocal broadcast on DVE. The NEXT
        chunk's qproj halves and the pending head-1 finalize are
        emitted inside each head's first score steps, where the AV
        pipeline hasn't started and PE would otherwise wait on exp.
  Proj: partial = outT.T @ Wo_i.T per [128,512] tile, lt-outer so each
        128-row band leaves as one 1MB DMA, PSUM evictions alternating
        between Scalar and Vector engines.
"""
import numpy as np

# ---- problem constants (hardcoded per contract) ----
B, L, D = 4, 1024, 1024
S, T = 2048, 1000
H, E = 16, 256
DL = 4096
BL = B * L            # 4096 query rows
EC = 512              # e-slice per core (2 heads)
NCORES = 8
TP = 1024             # T zero-padded (includes the v-bias ones row at 1000)

_CACHE = {}
MM_DTYPE = "bf16"     # "f32r" (safe, ~1e-4) or "bf16" (~1e-3, half the DMA)
AV_DELAY = 3          # AV matmuls trail the scores by this many steps


def _build_nc():
    from contextlib import ExitStack

    import concourse.tile as tile
    from concourse import bacc, mybir

    F32 = mybir.dt.float32
    F32R = mybir.dt.float32r
    MMD = mybir.dt.bfloat16 if MM_DTYPE == "bf16" else F32R
    AF = mybir.ActivationFunctionType
    MUL = mybir.AluOpType.mult
    ADD = mybir.AluOpType.add

    nc = bacc.Bacc("TRN2", target_bir_lowering=False, debug=False,
                   num_devices=NCORES)

    xt = nc.dram_tensor("xt", [D, BL], MMD, kind="ExternalInput")
    st = nc.dram_tensor("st", [TP, S], MMD, kind="ExternalInput")
    vt = nc.dram_tensor("vt", [TP, S], MMD, kind="ExternalInput")
    wqt = nc.dram_tensor("wqt", [D, EC], MMD, kind="ExternalInput")
    wkt = nc.dram_tensor("wkt", [TP, EC], MMD, kind="ExternalInput")
    wvt = nc.dram_tensor("wvt", [TP, EC], MMD, kind="ExternalInput")
    wot = nc.dram_tensor("wot", [EC, DL], MMD, kind="ExternalInput")
    bq_d = nc.dram_tensor("bq", [EC], F32, kind="ExternalInput")
    bk_d = nc.dram_tensor("bk", [EC], F32, kind="ExternalInput")
    out_d = nc.dram_tensor("out", [BL, DL], MMD, kind="ExternalOutput")

    NKD = 8            # k-tiles for D=1024
    NKT = 8            # k-tiles for TP=1024
    NLC = BL // 512    # 8 l-chunks
    NLT = BL // 128    # 32 l-tiles
    NST = S // 128     # 16 s-tiles
    NDC = DL // 512    # 8 out-proj chunks
    NSTP = NST // 2    # 8 score steps (2 s-tiles each)

    with tile.TileContext(nc) as tc:
        with ExitStack() as root:
            root.enter_context(
                nc.allow_low_precision(reason="bf16 matmul pipeline"))

            # ---- persistent pools ----
            consts = root.enter_context(tc.tile_pool(name="consts", bufs=1))
            kvp = root.enter_context(tc.tile_pool(name="kv", bufs=1))
            outp = root.enter_context(tc.tile_pool(name="outT", bufs=1))

            ones_f = consts.tile([128, 128], F32, name="ones_f")
            nc.vector.memset(ones_f[:], 1.0)
            ones_m = consts.tile([128, 128], MMD, name="ones_m")
            nc.vector.tensor_copy(ones_m[:], ones_f[:])
            bqbk_t = consts.tile([128, 8], F32, name="bqbk_t")
            bq_t = bqbk_t[:, 0:4]
            bk_t = bqbk_t[:, 4:8]

            # kT: 4 e-tiles x [128, S]; v: 4 tiles [128, 4*EC] (4 s-tiles each)
            kt_sb = [kvp.tile([128, S], MMD, name=f"kt{m}", tag=f"kt{m}")
                     for m in range(4)]
            v_sb = [kvp.tile([128, 4 * EC], MMD, name=f"v{g}", tag=f"v{g}")
                    for g in range(4)]
            # outT: 4 e-tiles x [128, BL]
            o_sb = [outp.tile([128, BL], MMD, name=f"oT{m}", tag=f"oT{m}")
                    for m in range(4)]

            # wq / wo / xq live in root pools; their loads are emitted
            # inside the KV phase so they prefetch during KV compute
            wq_pool = root.enter_context(tc.tile_pool(name="wq", bufs=1))
            wo_pool = root.enter_context(tc.tile_pool(name="wo", bufs=1))
            xq_pool = root.enter_context(tc.tile_pool(name="xq", bufs=2))
            wq_t = []
            wo_sb = wo_pool.tile([128, 4 * DL], MMD, name="wo_sb")

            xq_tiles = {}

            def load_xq(lc):
                t = xq_pool.tile([128, NKD * 512], MMD, tag="xq",
                                 name=f"xq{lc}")
                nc.sync.dma_start(
                    t[:].rearrange("p (k c) -> p k c", k=NKD),
                    xt.ap()[:, lc * 512:(lc + 1) * 512].rearrange(
                        "(k p) c -> p k c", p=128))
                xq_tiles[lc] = t

            # ---- phase KV: kT = Wk_i @ SE.T ; v = VE_aug @ Wv_aug ----
            # All contraction (T) tiles are grouped into single rearranged
            # DMAs ([128, k*(...)], 1KB runs) -- the chunked version was
            # gated on per-DMA issue cost on the Sync engine.
            with ExitStack() as ph:
                ph.enter_context(nc.named_scope("kvproj"))
                wkv_pool = ph.enter_context(tc.tile_pool(name="wkv", bufs=1))
                sk_pool = ph.enter_context(tc.tile_pool(name="sk", bufs=2))
                sv_pool = ph.enter_context(tc.tile_pool(name="sv", bufs=2))
                psk = ph.enter_context(
                    tc.tile_pool(name="psk", bufs=1, space="PSUM"))
                psv = ph.enter_context(
                    tc.tile_pool(name="psv", bufs=1, space="PSUM"))
                wk_sb = wkv_pool.tile([128, NKT * EC], MMD, name="wk_sb")
                wv_sb = wkv_pool.tile([128, NKT * EC], MMD, name="wv_sb")
                # wk (Sync queue) and st0 (Scalar queue, which starts
                # ~3us earlier) in quarter-DMAs so the first matmul waits
                # ~256KB per queue instead of 512KB
                def load_wk_q(qq):
                    nc.sync.dma_start(
                        wk_sb[:, qq * 2 * EC:(qq + 1) * 2 * EC].rearrange(
                            "p (k e) -> p k e", k=2),
                        wkt.ap()[qq * 256:(qq + 1) * 256, :].rearrange(
                            "(k p) e -> p k e", p=128))

                def load_st(sc, split=1, eng=None):
                    t = sk_pool.tile([128, NKT * 512], MMD, tag="stg",
                                     name=f"stg{sc}")
                    kh = NKT // split
                    for hf in range(split):
                        (eng or nc.sync).dma_start(
                            t[:, hf * kh * 512:(hf + 1) * kh * 512].rearrange(
                                "p (k c) -> p k c", k=kh),
                            st.ap()[hf * kh * 128:(hf + 1) * kh * 128,
                                    sc * 512:(sc + 1) * 512].rearrange(
                                "(k p) c -> p k c", p=128))
                    return t

                def load_vt(sc):
                    t = sv_pool.tile([128, NKT * 512], MMD, tag="vtg",
                                     name=f"vtg{sc}")
                    nc.sync.dma_start(
                        t[:].rearrange("p (k c) -> p k c", k=NKT),
                        vt.ap()[:, sc * 512:(sc + 1) * 512].rearrange(
                            "(k p) c -> p k c", p=128))
                    return t

                # st0 goes out on the (idle) Scalar engine's DMA queue, in
                # parallel with wk on the Sync queue
                st_t = load_st(0, split=4, eng=nc.scalar)
                for qq in range(4):
                    load_wk_q(qq)
                nc.sync.dma_start(bqbk_t[:, 0:4],
                                  bq_d.ap().rearrange("(m p) -> p m", p=128))
                nc.sync.dma_start(bqbk_t[:, 4:8],
                                  bk_d.ap().rearrange("(m p) -> p m", p=128))
                nc.sync.dma_start(
                    wv_sb[:].rearrange("p (k e) -> p k e", k=NKT),
                    wvt.ap().rearrange("(k p) e -> p k e", p=128))
                vt_t = load_vt(0)
                for sc in range(S // 512):
                    ps_k = [psk.tile([128, 512], F32, tag=f"psk{m}",
                                     name=f"psk{m}") for m in range(4)]
                    for kk in range(NKT):
                        for m in range(4):
                            nc.tensor.matmul(
                                ps_k[m][:],
                                wk_sb[:, kk * EC + m * 128:
                                      kk * EC + (m + 1) * 128],
                                st_t[:, kk * 512:(kk + 1) * 512],
                                start=(kk == 0), stop=(kk == NKT - 1))
                    if sc + 1 < S // 512:
                        st_next = load_st(sc + 1)
                    for m in range(4):
                        nc.scalar.activation(
                            kt_sb[m][:, sc * 512:(sc + 1) * 512], ps_k[m][:],
                            AF.Identity, bias=bk_t[:, m:m + 1])
                    # v group for the same 512-wide s range
                    g = sc
                    ps_v = [psv.tile([128, 512], F32, tag=f"psv{j}",
                                     name=f"psv{j}") for j in range(4)]
                    for kk in range(NKT):
                        for j in range(4):
                            nc.tensor.matmul(
                                ps_v[j][:],
                                vt_t[:, kk * 512 + j * 128:
                                     kk * 512 + (j + 1) * 128],
                                wv_sb[:, kk * EC:(kk + 1) * EC],
                                start=(kk == 0), stop=(kk == NKT - 1))
                    if sc + 1 < S // 512:
                        vt_next = load_vt(sc + 1)
                    # wq/xq0/wo prefetches spread across the sc loop, always
                    # emitted AFTER the next sc's st/vt loads so they never
                    # delay the KV pipeline (they aren't needed until the
                    # attn/proj phases)
                    if sc == 0:
                        wq_sb = wq_pool.tile([128, NKD * EC], MMD,
                                             name="wq_sb")
                        nc.sync.dma_start(
                            wq_sb[:].rearrange("p (k e) -> p k e", k=NKD),
                            wqt.ap().rearrange("(k p) e -> p k e", p=128))
                        wq_t.append(wq_sb)
                    elif sc == 1:
                        load_xq(0)
                        nc.sync.dma_start(wo_sb[:, 0:DL], wot[0:128, :])
                    elif sc == 2:
                        for ke in (1, 2):
                            nc.sync.dma_start(
                                wo_sb[:, ke * DL:(ke + 1) * DL],
                                wot[ke * 128:(ke + 1) * 128, :])
                    else:
                        nc.sync.dma_start(wo_sb[:, 3 * DL:4 * DL],
                                          wot[3 * 128:4 * 128, :])
                    for j in range(4):
                        nc.scalar.activation(
                            v_sb[g][:, j * EC:(j + 1) * EC], ps_v[j][:],
                            AF.Copy)
                    if sc + 1 < S // 512:
                        st_t, vt_t = st_next, vt_next

            # ---- fused attention phase (q-projection + attention per lc) ----
            with ExitStack() as ph:
                ph.enter_context(nc.named_scope("attn"))
                qt_pool = ph.enter_context(tc.tile_pool(name="qtp", bufs=2))
                a_pool = ph.enter_context(tc.tile_pool(name="ap", bufs=1))
                acc_pool = ph.enter_context(tc.tile_pool(name="accp", bufs=2))
                bc_pool = ph.enter_context(tc.tile_pool(name="bcp", bufs=2))
                # PSUM budget (8 banks): misc (qproj + denom) 2, scores
                # double-buffered 4, attn-out accumulators 2.
                ps_misc_p = ph.enter_context(
                    tc.tile_pool(name="ps_misc", bufs=2, space="PSUM"))
                ps_sT_p = ph.enter_context(
                    tc.tile_pool(name="ps_sT", bufs=2, space="PSUM"))
                ps_o_p = ph.enter_context(
                    tc.tile_pool(name="ps_o", bufs=2, space="PSUM"))

                def qproj_half(lc, half, qt_t):
                    # qt_t is the per-head tile: tile-granular dep tracking
                    # means head 0's scores would otherwise wait on head 1's
                    # bias evictions
                    xq_t = xq_tiles[lc]
                    # the two 8-matmul chains run back-to-back (not
                    # interleaved) so the first chain's bias-eviction
                    # overlaps the second chain instead of gating the
                    # scores after the whole qproj
                    for mh in range(2):
                        m = half * 2 + mh
                        ps_q = ps_misc_p.tile([128, 512], F32, tag="m",
                                              name=f"psq{mh}")
                        for kk in range(NKD):
                            nc.tensor.matmul(
                                ps_q[:],
                                wq_t[0][:, kk * EC + m * 128:
                                      kk * EC + (m + 1) * 128],
                                xq_t[:, kk * 512:(kk + 1) * 512],
                                start=(kk == 0), stop=(kk == NKD - 1))
                        nc.scalar.activation(
                            qt_t[:, mh * 512:(mh + 1) * 512], ps_q[:],
                            AF.Identity, bias=bq_t[:, m:m + 1])

                def attn_head_main(lc, h, qt_t, a_t, inserts=None):
                    # scores in double-buffered 2-bank PSUM tiles; one exp
                    # per 1024 columns; each score step owns its own a-tile
                    # so exp never false-WARs against the DVE accumulation
                    # of the previous step. AV matmuls are software-
                    # pipelined AV_DELAY steps behind the scores so PE
                    # keeps working while ACT runs the exps.
                    acc = acc_pool.tile([128, 1024], F32, tag="acc",
                                        name="acc")
                    ps_os = [ps_o_p.tile([128, 512], F32, tag="ps_o",
                                         name="ps_o") for _ in range(2)]

                    def av_pair(stp):
                        for et in range(2):
                            for sub in range(2):
                                stt = 2 * stp + sub
                                nc.tensor.matmul(
                                    ps_os[et][:],
                                    v_sb[stt // 4][:, (stt % 4) * EC + h * E
                                                   + et * 128:
                                                   (stt % 4) * EC + h * E
                                                   + (et + 1) * 128],
                                    a_t[stp][:, sub * 512:(sub + 1) * 512],
                                    start=(stt == 0), stop=(stt == NST - 1))

                    for stp in range(NSTP):
                        ps_sT = ps_sT_p.tile([128, 1024], F32, tag="ps_sT",
                                             name="ps_sT")
                        for sub in range(2):
                            stt = 2 * stp + sub
                            for et in range(2):
                                m = 2 * h + et
                                nc.tensor.matmul(
                                    ps_sT[:, sub * 512:(sub + 1) * 512],
                                    kt_sb[m][:, stt * 128:(stt + 1) * 128],
                                    qt_t[:, et * 512:(et + 1) * 512],
                                    start=(et == 0), stop=(et == 1))
                        a_ap = a_t[stp][:]
                        nc.scalar.activation(a_ap, ps_sT[:], AF.Exp,
                                             scale=0.0625)
                        # accumulate denominator on DVE
                        if stp == 0:
                            nc.vector.tensor_copy(acc[:], a_ap)
                        else:
                            nc.vector.tensor_tensor(acc[:], acc[:], a_ap,
                                                    ADD)
                        # exp-independent PE work (next chunk's qproj, the
                        # pending finalize) fills the ramp bubble where the
                        # AV pipeline hasn't started yet
                        if inserts and stp in inserts:
                            inserts[stp]()
                        if stp >= AV_DELAY:
                            av_pair(stp - AV_DELAY)
                    for stp in range(NSTP - AV_DELAY, NSTP):
                        av_pair(stp)
                    return acc, ps_os

                def attn_fin(lc, h, acc, ps_os):
                    # softmax denominators: fold acc halves (bf16 out, so
                    # the reduce matmul stays in bf16 -- no PE fp32-mode
                    # switch), then a single ones-matrix matmul performs
                    # the partition reduction AND the broadcast (every
                    # output row = total sum); reciprocal then runs on all
                    # 128 partitions.
                    accb = bc_pool.tile([128, 512], MMD, tag="accb",
                                        name="accb")
                    nc.vector.tensor_tensor(accb[:], acc[:, 0:512],
                                            acc[:, 512:1024], ADD)
                    ps_b = ps_misc_p.tile([128, 512], F32, tag="m",
                                          name="ps_b")
                    nc.tensor.matmul(ps_b[:], ones_m[:], accb[:],
                                     start=True, stop=True)
                    bc = bc_pool.tile([128, 512], F32, tag="bc", name="bc")
                    nc.vector.reciprocal_approx_fast(out=bc[:], in_=ps_b[:])
                    for et in range(2):
                        m = 2 * h + et
                        nc.vector.tensor_tensor(
                            o_sb[m][:, lc * 512:(lc + 1) * 512],
                            ps_os[et][:], bc[:], MUL)

                # Cross-phase software pipeline: the NEXT chunk's qproj
                # halves and the pending head-1 finalize are emitted inside
                # each head's ramp (steps 0-1, where the AV pipeline hasn't
                # started and PE would otherwise stall on exp draining the
                # scores PSUM).
                qt_tiles = {}

                def make_qt(lc):
                    qt_tiles[lc] = [
                        qt_pool.tile([128, 2 * 512], MMD, tag=f"qt{h}",
                                     name=f"qt{h}_{lc}") for h in range(2)]

                pending = None
                make_qt(0)
                qproj_half(0, 0, qt_tiles[0][0])
                qproj_half(0, 1, qt_tiles[0][1])
                for lc in range(NLC):
                    a_t = [a_pool.tile([128, 1024], MMD, tag=f"a{g}",
                                       name=f"a{g}") for g in range(NSTP)]
                    if lc + 1 < NLC:
                        load_xq(lc + 1)
                        make_qt(lc + 1)
                    ins0 = {}
                    if pending is not None:
                        fa = pending
                        ins0[0] = lambda fa=fa: attn_fin(*fa)
                    if lc + 1 < NLC:
                        ins0[1] = lambda l=lc + 1: qproj_half(
                            l, 0, qt_tiles[l][0])
                    r0 = attn_head_main(lc, 0, qt_tiles[lc][0], a_t, ins0)
                    attn_fin(lc, 0, *r0)
                    ins1 = {}
                    if lc + 1 < NLC:
                        ins1[1] = lambda l=lc + 1: qproj_half(
                            l, 1, qt_tiles[l][1])
                    r1 = attn_head_main(lc, 1, qt_tiles[lc][1], a_t, ins1)
                    pending = (lc, 1, *r1)
                attn_fin(*pending)

            # ---- out-projection: partial = outT.T @ WoT -> DRAM ----
            # lt-outer so each 128-row band finishes as one contiguous
            # [128, 4096] bf16 tile -> a single 1MB output DMA per band
            with ExitStack() as ph:
                ph.enter_context(nc.named_scope("proj"))
                pev_pool = ph.enter_context(tc.tile_pool(name="pev", bufs=4))
                psp = ph.enter_context(
                    tc.tile_pool(name="psp", bufs=4, space="PSUM"))
                for lt in range(NLT):
                    ev = pev_pool.tile([128, DL], MMD, tag="pev",
                                       name="pev")
                    for dc in range(NDC):
                        ps_p = psp.tile([128, 512], F32, tag="ps_p",
                                        name="ps_p")
                        for ke in range(4):
                            nc.tensor.matmul(
                                ps_p[:],
                                o_sb[ke][:, lt * 128:(lt + 1) * 128],
                                wo_sb[:, ke * DL + dc * 512:
                                      ke * DL + (dc + 1) * 512],
                                start=(ke == 0), stop=(ke == 3))
                        # split PSUM eviction across the two idle engines
                        if dc % 2 == 0:
                            nc.vector.tensor_copy(
                                ev[:, dc * 512:(dc + 1) * 512], ps_p[:])
                        else:
                            nc.scalar.activation(
                                ev[:, dc * 512:(dc + 1) * 512], ps_p[:],
                                AF.Copy)
                        if lt == NLT - 1:
                            # last band: per-chunk DMAs right behind each
                            # eviction so the flush after the final matmul
                            # is 128KB instead of 1MB
                            nc.sync.dma_start(
                                out_d[lt * 128:(lt + 1) * 128,
                                      dc * 512:(dc + 1) * 512],
                                ev[:, dc * 512:(dc + 1) * 512])
                    if lt < NLT - 1:
                        nc.sync.dma_start(out_d[lt * 128:(lt + 1) * 128, :],
                                          ev[:])

    nc.compile()
    return nc


def _get_nc():
    if "nc" not in _CACHE:
        _CACHE["nc"] = _build_nc()
    return _CACHE["nc"]


def _build_in_maps(inputs):
    return _prep(**{k: inputs[k] for k in (
        "target_embedding", "source_embedding", "value_embedding",
        "Wq", "bq", "Wk", "bk", "Wv", "bv", "Wo")})


def _prep(target_embedding, source_embedding, value_embedding,
          Wq, bq, Wk, bk, Wv, bv, Wo):
    if MM_DTYPE == "bf16":
        import ml_dtypes
        mmd = ml_dtypes.bfloat16
    else:
        mmd = np.float32
    f32 = np.float32
    X = np.asarray(target_embedding, f32).reshape(BL, D)
    xt = np.ascontiguousarray(X.T)                       # [D, BL]
    st = np.zeros((TP, S), f32)
    st[:T] = np.asarray(source_embedding, f32).T
    vt = np.zeros((TP, S), f32)
    vt[:T] = np.asarray(value_embedding, f32).T
    vt[T] = 1.0                                          # v-bias ones row
    WqT = np.asarray(Wq, f32).T                          # [D, H*E]
    WkT = np.asarray(Wk, f32).T                          # [T, H*E]
    WvT = np.asarray(Wv, f32).T                          # [T, H*E]
    WoT = np.asarray(Wo, f32).T                          # [H*E, DL]
    bq = np.asarray(bq, f32)
    bk = np.asarray(bk, f32)
    bv = np.asarray(bv, f32)

    xt_c = xt.astype(mmd)
    st_c = st.astype(mmd)
    vt_c = vt.astype(mmd)
    in_maps = []
    for i in range(NCORES):
        sl = slice(i * EC, (i + 1) * EC)
        wkt_i = np.zeros((TP, EC), f32)
        wkt_i[:T] = WkT[:, sl]
        wvt_i = np.zeros((TP, EC), f32)
        wvt_i[:T] = WvT[:, sl]
        wvt_i[T] = bv[sl]
        in_maps.append({
            "xt": xt_c,
            "st": st_c,
            "vt": vt_c,
            "wqt": np.ascontiguousarray(WqT[:, sl]).astype(mmd),
            "wkt": wkt_i.astype(mmd),
            "wvt": wvt_i.astype(mmd),
            "wot": np.ascontiguousarray(WoT[sl, :]).astype(mmd),
            "bq": np.ascontiguousarray(bq[sl]),
            "bk": np.ascontiguousarray(bk[sl]),
        })
    return in_maps


def kernel(target_embedding, source_embedding, value_embedding,
           Wq, bq, Wk, bk, Wv, bv, Wo, bo):
    from concourse.bass_utils import run_bass_kernel_spmd

    in_maps = _prep(target_embedding, source_embedding, value_embedding,
                    Wq, bq, Wk, bk, Wv, bv, Wo)
    _CACHE["in_maps"] = in_maps
    nc = _get_nc()
    res = run_bass_kernel_spmd(nc, in_maps, list(range(NCORES)))

    acc = np.zeros((BL, DL), np.float32)
    for i in range(NCORES):
        acc += np.asarray(res.results[i]["out"]).astype(np.float32)
    out = (acc + np.asarray(bo, np.float32)[None, :]).astype(np.float32)
    return out.reshape(B, L, DL)
